# revision 3
# baseline (speedup 1.0000x reference)
"""Trainium2 Bass kernel for per-class mean soft-target cross-entropy.

Reference computation (see problem):
    y_cls  = argmax(y, axis=1)                      # [B]
    loss_i = -sum_c y[i,c] * log_softmax(y_hat)[i,c]
           = lse_i * sy_i - dot_i
      with lse_i = log(sum_c exp(y_hat[i,c])), sy_i = sum_c y[i,c],
           dot_i = sum_c y[i,c]*y_hat[i,c]
    out[c] = mean of loss_i over rows with y_cls == c  (0 if empty)

Strategy (8 cores, data-parallel over the batch):
  Each core processes 62464 rows (61 blocks of 1024 rows); the 36-row
  tail per core is computed on the host (288 rows of 500000 total).
  Per 1024-row block (rows live on the 128 partitions, 8 rows per
  partition, contiguous 512KB DMAs):
    ACT : e = exp(y_hat)  (batched, bf16 out)
          lse = Ln(sum_c e)
          yl_j = y_j * lse_j  (Copy activation with per-partition scale)
    DVE : sexp = reduce_sum(e), m_y = reduce_max(y), plus small splits
    GPS : onehot = is_equal(y, broadcast(m_y)), most of P = y*y_hat
    PE  : psum[c, :] += onehot_j^T @ [P_j | yl_j | 1]   (257 columns)
  After 61 blocks the PSUM [128, 257] holds, per class c:
    cols 0:128   sum over class members of y*y_hat contributions (seg_dot)
    cols 128:256 sum of y*lse contributions (seg_lse_sy)
    col  256     member count
  The host reduces the 8 per-core [128,257] dumps, adds the exact tail
  rows, corrects argmax ties (equality one-hot counts every tied class;
  the reference argmax takes the first), and divides.
"""

import numpy as np
from contextlib import ExitStack

# ---------------------------------------------------------------- config
N_CORES = 8
B_TOTAL = 500000
C = 128                      # classes
T = 8                        # 128-row tiles per block (rows per partition)
BLOCK_ROWS = 128 * T         # 1024
N_BLOCKS = 61
K_ROWS = N_BLOCKS * BLOCK_ROWS   # 62464 rows through the kernel per core
RPC = B_TOTAL // N_CORES         # 62500 rows owned per core
N_COLS = 2 * C + 1               # 257 psum columns

# engine splits (tunable): which j-tiles each engine handles.
# NOTE: Pool/GpSimd cannot encode TensorTensor/TensorScalar on TRN2
# (walrus ISA check), so all ALU work lives on DVE + ACT.
P_J_GP = []                      # y*y_hat multiply on GpSimd (unsupported)
P_J_DVE = list(range(0, 8))      # ... on Vector (batched)
YL_J_ACT = list(range(0, 8))     # y*lse scale on Scalar(ACT)
YL_J_DVE = []                    # ... and on Vector
CMP_ENGINE = "vector"            # one-hot compare engine

_BUILT = None


def _build_nc(n_blocks=N_BLOCKS):
    import concourse.tile as tile
    from concourse import bacc, mybir

    f32 = mybir.dt.float32
    bf16 = mybir.dt.bfloat16
    OP = mybir.AluOpType
    AF = mybir.ActivationFunctionType
    X = mybir.AxisListType.X

    k_rows = n_blocks * BLOCK_ROWS
    nc = bacc.Bacc(
        "TRN2",
        target_bir_lowering=False,
        debug=False,
        num_devices=N_CORES,
    )
    yh_d = nc.dram_tensor("y_hat", [k_rows, C], f32, kind="ExternalInput").ap()
    y_d = nc.dram_tensor("y", [k_rows, C], f32, kind="ExternalInput").ap()
    out_d = nc.dram_tensor("out", [C, N_COLS], f32, kind="ExternalOutput").ap()

    # row r = b*1024 + p*8 + j  ->  block b, partition p, slot j
    yh_b = yh_d.rearrange("(b p j) c -> b p j c", p=128, j=T)
    y_b = y_d.rearrange("(b p j) c -> b p j c", p=128, j=T)

    with tile.TileContext(nc) as tc, ExitStack() as ctx:
        io = ctx.enter_context(tc.tile_pool(name="io", bufs=4))
        ohp = ctx.enter_context(tc.tile_pool(name="ohp", bufs=3))
        ep = ctx.enter_context(tc.tile_pool(name="ep", bufs=3))
        st = ctx.enter_context(tc.tile_pool(name="st", bufs=4))
        mm = ctx.enter_context(tc.tile_pool(name="mm", bufs=1))
        ps = ctx.enter_context(tc.tile_pool(name="ps", bufs=1, space="PSUM"))

        psum = ps.tile([C, N_COLS], f32)

        # two persistent moving-operand tiles; the constant ones column is
        # written once and survives because later blocks only overwrite the
        # P and yl column groups.
        Ms = [
            mm.tile([128, T, N_COLS], bf16, tag=f"M{i}", name=f"M{i}")
            for i in range(2)
        ]
        for Mt in Ms:
            nc.vector.memset(Mt[:, :, 2 * C], 1.0)

        for b in range(n_blocks):
            yh = io.tile([128, T, C], f32, tag="yh")
            y = io.tile([128, T, C], f32, tag="y")
            nc.sync.dma_start(yh, yh_b[b])
            nc.sync.dma_start(y, y_b[b])

            M = Ms[b % 2]

            # --- ACT: exp (batched over the whole block), bf16 out
            e = ep.tile([128, T, C], bf16, tag="e")
            nc.scalar.activation(e, yh, AF.Exp)

            # --- DVE: row sums of exp, row max of y
            sexp = st.tile([128, T], f32, tag="sexp")
            nc.vector.tensor_reduce(sexp, e, axis=X, op=OP.add)
            m_y = st.tile([128, T], f32, tag="m_y")
            nc.vector.tensor_reduce(m_y, y, axis=X, op=OP.max)

            # --- ACT: lse = Ln(sum exp)
            lse = st.tile([128, T], f32, tag="lse")
            nc.scalar.activation(lse, sexp, AF.Ln)

            # --- one-hot: y == rowmax (broadcast along the class dim)
            oh = ohp.tile([128, T, C], bf16, tag="oh")
            cmp_eng = nc.gpsimd if CMP_ENGINE == "gpsimd" else nc.vector
            cmp_eng.tensor_tensor(
                oh, y, m_y.broadcast_to([128, T, C]), op=OP.is_equal
            )

            # --- P = y * y_hat into M cols 0:C (split across engines)
            if P_J_GP:
                j0, j1 = P_J_GP[0], P_J_GP[-1] + 1
                nc.gpsimd.tensor_tensor(
                    M[:, j0:j1, 0:C], y[:, j0:j1, :], yh[:, j0:j1, :], op=OP.mult
                )
            if P_J_DVE:
                j0, j1 = P_J_DVE[0], P_J_DVE[-1] + 1
                nc.vector.tensor_tensor(
                    M[:, j0:j1, 0:C], y[:, j0:j1, :], yh[:, j0:j1, :], op=OP.mult
                )

            # --- yl = y * lse into M cols C:2C (per-tile, per-partition scale)
            for j in YL_J_ACT:
                nc.scalar.activation(
                    M[:, j, C : 2 * C],
                    y[:, j, :],
                    AF.Copy,
                    bias=0.0,
                    scale=lse[:, j : j + 1],
                )
            for j in YL_J_DVE:
                nc.vector.tensor_scalar(
                    out=M[:, j, C : 2 * C],
                    in0=y[:, j, :],
                    scalar1=lse[:, j : j + 1],
                    scalar2=None,
                    op0=OP.mult,
                )

            # --- PE: accumulate per-class sums
            for j in range(T):
                nc.tensor.matmul(
                    psum,
                    oh[:, j, :],
                    M[:, j, :],
                    start=(b == 0 and j == 0),
                    stop=(b == n_blocks - 1 and j == T - 1),
                )

        res = st.tile([C, N_COLS], f32, tag="res")
        nc.vector.tensor_copy(res, psum)
        nc.sync.dma_start(out_d, res)

    nc.compile()
    return nc


def _get_built():
    global _BUILT
    if _BUILT is None:
        _BUILT = _build_nc()
    return _BUILT


# ------------------------------------------------------------- host math
def _host_loss(y_hat_rows, y_rows):
    """Exact per-row loss + first-argmax class, in float64."""
    yh = y_hat_rows.astype(np.float64)
    y = y_rows.astype(np.float64)
    m = yh.max(axis=1, keepdims=True)
    lse = (m + np.log(np.exp(yh - m).sum(axis=1, keepdims=True)))[:, 0]
    loss = lse * y.sum(axis=1) - (y * yh).sum(axis=1)
    cls = y_rows.argmax(axis=1)  # first max, matching the reference
    return cls, loss


def kernel(y_hat, y):
    from concourse.bass_utils import run_bass_kernel_spmd

    y_hat = np.asarray(y_hat, dtype=np.float32)
    y = np.asarray(y, dtype=np.float32)
    assert y_hat.shape == (B_TOTAL, C) and y.shape == (B_TOTAL, C)

    nc = _get_built()
    in_maps = []
    for c in range(N_CORES):
        r0 = c * RPC
        in_maps.append(
            {
                "y_hat": np.ascontiguousarray(y_hat[r0 : r0 + K_ROWS]),
                "y": np.ascontiguousarray(y[r0 : r0 + K_ROWS]),
            }
        )
    res = run_bass_kernel_spmd(nc, in_maps, core_ids=list(range(N_CORES)))
    outs = np.stack([r["out"] for r in res.results]).astype(np.float64)  # [8,128,257]

    seg_dot = outs[:, :, 0:C].sum(axis=(0, 2))
    seg_ylse = outs[:, :, C : 2 * C].sum(axis=(0, 2))
    counts = outs[:, :, 2 * C].sum(axis=0)
    seg_sum = seg_ylse - seg_dot

    # --- tail rows not covered by the kernel (36 per core)
    tail_idx = np.concatenate(
        [np.arange(c * RPC + K_ROWS, (c + 1) * RPC) for c in range(N_CORES)]
    )
    if tail_idx.size:
        tcls, tloss = _host_loss(y_hat[tail_idx], y[tail_idx])
        np.add.at(seg_sum, tcls, tloss)
        np.add.at(counts, tcls, 1.0)

    # --- argmax-tie correction: the device one-hot credits every class tied
    # at the row max; the reference argmax credits only the first.
    kmask = np.zeros(B_TOTAL, dtype=bool)
    for c in range(N_CORES):
        kmask[c * RPC : c * RPC + K_ROWS] = True
    ymax = y.max(axis=1, keepdims=True)
    nmax = (y == ymax).sum(axis=1)
    ties = np.flatnonzero((nmax > 1) & kmask)
    if ties.size:
        _, tie_loss = _host_loss(y_hat[ties], y[ties])
        for row, li in zip(ties, tie_loss):
            cls_all = np.flatnonzero(y[row] == ymax[row, 0])
            for cdup in cls_all[1:]:
                counts[cdup] -= 1.0
                seg_sum[cdup] -= li

    out = np.where(counts > 0, seg_sum / np.maximum(counts, 1.0), 0.0)
    return out.astype(np.float32)
